# revision 30
# baseline (speedup 1.0000x reference)
"""Trainium2 Bass kernel for nn_DE_TGraph (diachronic temporal-graph GNN layer).

Strategy (8 NeuronCores, SPMD, relation-sharded):
  - 460 relations partitioned across 8 cores by size-rank snake order
    (rank group g -> slot g; slot capacity = r16(max count in group), so
    pad waste is small and the slot structure is uniform across cores).
    Per-relation BatchNorm is core-local (no collectives for BN stats).
  - Per-neighbor features gathered with ONE dma_gather from a host-packed
    table [ent(96) | yf mf df | yp mp dp | ya ma da] (+ zero row); within
    each slot neighbors are sorted by entity id for HBM locality.
  - Time embedding on DVE/ACT (sin on ScalarE), x tiles transposed on PE.
  - GEMM: two passes over slots (stationary W per slot). Pass 1 feeds
    bn_stats (per-slot mean/var); pass 2 recomputes z and applies the
    per-relation BN affine + ReLU via one ScalarE activation per slot
    (scale/bias are per-partition columns), writing z^T directly to SBUF.
  - Pooling + scoring fused: only edges whose dst is referenced by
    head_pos/tail_pos are kept (~41%).  Host builds per-128-edge-tile
    matrices P[e,b] = (1[dst_e==head_b] - 1[dst_e==tail_b]) / pcnt[dst_e].
    Device: z rows -> DRAM, dma_gather by edge src, then
    D^T[d,b] += zg_t^T @ P_t accumulated in PSUM.  D^T is each core's
    partial of emb[head]-emb[tail] for all B triples.
  - ONE small AllReduce (8 cores) of D^T [128, B] bf16 (256 KB) with a
    Shared-addr-space output (fast HBM-HBM collective path).
  - Scoring: v = D^T + rel_embs[rels]^T (host-gathered, transposed);
    score = -sqrt(sum_d v^2) via square + ones-vector matmul reduction
    over partitions; output written as [1, B].
"""
import numpy as np

import concourse.bacc as bacc
import concourse.mybir as mybir
import concourse.tile as tile
from concourse import library_config
from concourse.bass_utils import run_bass_kernel_spmd

F32 = mybir.dt.float32
BF16 = mybir.dt.bfloat16
USE_BF16 = True
DT16 = BF16 if USE_BF16 else F32
import ml_dtypes
NPDT16 = ml_dtypes.bfloat16 if USE_BF16 else np.float32
I16 = mybir.dt.int16
AF = mybir.ActivationFunctionType
OP = mybir.AluOpType

NUM_ENT = 10000
NUM_REL = 230
R2 = 2 * NUM_REL
S_DIM = 96
T_DIM = 32
DIM = 128
N = 32768
Q = 4096
E = 32768
B = 1024
NC = 8
BN_EPS = 1e-5


def _r16(x):
    return (int(x) + 15) // 16 * 16


def _r128(x):
    return (int(x) + 127) // 128 * 128


def _wrap16(idx, n):
    """[n] int -> [128, n//16] int16 (index i at [i%16, i//16], replicated)."""
    assert n % 16 == 0 and len(idx) == n
    w = np.asarray(idx).reshape(n // 16, 16).T.astype(np.int16)
    return np.ascontiguousarray(np.tile(w, (8, 1)))


def _tile_rows(x, p=128):
    """[n, d] -> [128, n//p, d] tile layout (row c*128+p -> [p, c, :])."""
    n, d = x.shape
    assert n % p == 0
    return np.ascontiguousarray(x.reshape(n // p, p, d).transpose(1, 0, 2))


class _Plan:
    pass


def _make_plan(inp):
    p = _Plan()
    rel = np.asarray(inp["rel_id"]).astype(np.int64).reshape(-1)
    nidx = np.asarray(inp["neighbor_idx"]).astype(np.int64).reshape(-1)
    years = np.asarray(inp["years"], np.float32).reshape(-1)
    months = np.asarray(inp["months"], np.float32).reshape(-1)
    days = np.asarray(inp["days"], np.float32).reshape(-1)
    psrc = np.asarray(inp["pool_src"]).astype(np.int64).reshape(-1)
    pdst = np.asarray(inp["pool_dst"]).astype(np.int64).reshape(-1)
    head = np.asarray(inp["head_pos"]).astype(np.int64).reshape(-1)
    tail = np.asarray(inp["tail_pos"]).astype(np.int64).reshape(-1)
    rels = np.asarray(inp["rels"]).astype(np.int64).reshape(-1)
    W = np.asarray(inp["W"], np.float32)
    b = np.asarray(inp["b"], np.float32)
    gamma = np.asarray(inp["gamma"], np.float32)
    beta = np.asarray(inp["beta"], np.float32)

    # ---- relation partition: size-rank snake across cores ----
    cnts = np.bincount(rel, minlength=R2)
    order = np.argsort(-cnts, kind="stable")
    SLOTS = (R2 + NC - 1) // NC
    core_rels = [[None] * SLOTS for _ in range(NC)]
    caps = np.zeros(SLOTS, np.int64)
    for s in range(SLOTS):
        grp = order[s * NC : (s + 1) * NC]
        caps[s] = cnts[grp[0]]  # exact group max; only the total is padded
        perm = range(NC) if s % 2 == 0 else range(NC - 1, -1, -1)
        for r, c in zip(grp, perm):
            core_rels[c][s] = int(r)
    assert caps.max() <= 512, "relation bucket exceeds 512 (unsupported)"
    col_start = np.concatenate([[0], np.cumsum(caps)])
    NCOLS = int(col_start[-1])
    N_cap = _r128(NCOLS)
    C_n = N_cap // 128

    # positions per relation
    order_by_rel = np.argsort(rel, kind="stable")
    rel_sorted = rel[order_by_rel]
    starts = np.searchsorted(rel_sorted, np.arange(R2))
    ends = np.searchsorted(rel_sorted, np.arange(R2), side="right")

    pos_core = np.empty(N, np.int64)
    pos_col = np.empty(N, np.int64)
    gidx_cores, tcat_cores = [], []
    for c in range(NC):
        gidx = np.full(N_cap, NUM_ENT, np.int64)  # zero row
        tc3 = np.zeros((N_cap, 3), np.float32)
        for s in range(SLOTS):
            r = core_rels[c][s]
            if r is None:
                continue
            pp = order_by_rel[starts[r] : ends[r]]
            pp = pp[np.argsort(nidx[pp], kind="stable")]  # HBM locality
            j0 = int(col_start[s])
            gidx[j0 : j0 + len(pp)] = nidx[pp]
            tc3[j0 : j0 + len(pp), 0] = years[pp]
            tc3[j0 : j0 + len(pp), 1] = months[pp]
            tc3[j0 : j0 + len(pp), 2] = days[pp]
            pos_core[pp] = c
            pos_col[pp] = j0 + np.arange(len(pp))
        gidx_cores.append(_wrap16(gidx, N_cap))
        tcat_cores.append(_tile_rows(tc3).astype(NPDT16))

    # ---- per-core weights + BN aux ----
    wloc_cores, aux_cores = [], []
    for c in range(NC):
        wl = np.zeros((SLOTS, DIM, DIM), np.float32)
        invcnt = np.ones(SLOTS, np.float32)
        onem = np.ones(SLOTS, np.float32)
        gT_u = np.zeros((DIM, SLOTS), np.float32)
        ubb = np.zeros((DIM, SLOTS), np.float32)
        for s in range(SLOTS):
            r = core_rels[c][s]
            if r is None:
                continue
            wl[s] = W[r]
            cnt = cnts[r]
            invcnt[s] = 1.0 / max(cnt, 1)
            u = 1.0 if cnt > 1 else 0.0
            onem[s] = 1.0 - u
            gT_u[:, s] = gamma[r] * u
            ubb[:, s] = beta[r] * u + b[r] * (1.0 - u)
        aux = np.zeros((128, 4 * SLOTS), np.float32)
        aux[:, 0:SLOTS] = invcnt[None, :]
        aux[:, SLOTS : 2 * SLOTS] = onem[None, :]
        aux[:, 2 * SLOTS : 3 * SLOTS] = gT_u
        aux[:, 3 * SLOTS : 4 * SLOTS] = ubb
        wloc_cores.append(np.ascontiguousarray(
            wl.transpose(1, 0, 2).astype(NPDT16)))
        aux_cores.append(np.ascontiguousarray(aux))

    # ---- pooling edges: keep only dsts referenced by head/tail ----
    pcnt = np.bincount(pdst, minlength=Q).astype(np.float32)
    used = np.zeros(Q, bool)
    used[head] = True
    used[tail] = True
    keep = used[pdst]
    e_core = pos_core[psrc]
    ecols, edsts = [], []
    for c in range(NC):
        m = keep & (e_core == c)
        es, ed = psrc[m], pdst[m]
        o = np.argsort(pos_col[es], kind="stable")  # HBM locality
        ecols.append(pos_col[es[o]])
        edsts.append(ed[o])
    T_E = max(1, max((len(x) + 127) // 128 for x in ecols))
    NE = T_E * 128

    egidx_cores, p_cores = [], []
    tile_ready = np.zeros(T_E, np.int64)  # z_d chunks needed per edge tile
    for c in range(NC):
        ec, ed = ecols[c], edsts[c]
        L = len(ec)
        eg = np.zeros(NE, np.int64)
        eg[:L] = ec
        dl = np.full(NE, -1, np.int64)
        dl[:L] = ed
        inv = np.zeros(NE, np.float32)
        inv[:L] = 1.0 / np.maximum(pcnt[ed], 1.0)
        P = ((dl[:, None] == head[None, :]).astype(np.float32)
             - (dl[:, None] == tail[None, :]).astype(np.float32))
        P *= inv[:, None]
        egidx_cores.append(_wrap16(eg, NE))
        p_cores.append(np.ascontiguousarray(
            P.reshape(T_E, 128, B).transpose(1, 0, 2).astype(NPDT16)))
        hi = eg.reshape(T_E, 128).max(axis=1)  # cols sorted -> per-tile max
        tile_ready = np.maximum(tile_ready, hi // 128 + 1)
    p.tile_ready = tuple(int(x) for x in tile_ready)

    # ---- scoring: rel_embs[rels]^T / NC, folded pre-AllReduce ----
    relgT = np.asarray(inp["rel_embs"], np.float32)[rels].T / NC  # [128, B]
    p.relgT = np.ascontiguousarray(relgT.astype(NPDT16))
    p.ident = np.ascontiguousarray(np.eye(128, dtype=NPDT16))

    # ---- packed entity/time table (replicated) ----
    tbl = np.zeros((NUM_ENT + 1, 384), np.float32)
    tbl[:NUM_ENT, 0:96] = np.asarray(inp["ent_embs"], np.float32)
    for k, nm in enumerate(["y_freq", "m_freq", "d_freq",
                            "y_phi", "m_phi", "d_phi",
                            "y_amp", "m_amp", "d_amp"]):
        tbl[:NUM_ENT, 96 + 32 * k : 128 + 32 * k] = np.asarray(inp[nm], np.float32)
    p.tbl = np.ascontiguousarray(tbl.astype(NPDT16))

    p.SLOTS, p.caps, p.col_start = SLOTS, caps, col_start
    p.NCOLS, p.N_cap, p.C_n, p.T_E = NCOLS, N_cap, C_n, T_E
    p.gidx, p.tcat = gidx_cores, tcat_cores
    p.wloc, p.aux = wloc_cores, aux_cores
    p.egidx, p.P = egidx_cores, p_cores
    return p


def _build(SLOTS, caps, col_start, NCOLS, N_cap, C_n, T_E, tile_ready):
    nc = bacc.Bacc(None, target_bir_lowering=False, debug=False, num_devices=NC)
    tbl = nc.dram_tensor("tbl", [NUM_ENT + 1, 384], DT16, kind="ExternalInput")
    gidx = nc.dram_tensor("gidx", [128, N_cap // 16], I16, kind="ExternalInput")
    tcat = nc.dram_tensor("tcat", [128, C_n, 3], DT16, kind="ExternalInput")
    wloc = nc.dram_tensor("wloc", [128, SLOTS, DIM], DT16, kind="ExternalInput")
    aux = nc.dram_tensor("aux", [128, 4 * SLOTS], F32, kind="ExternalInput")
    egidx = nc.dram_tensor("egidx", [128, T_E * 128 // 16], I16,
                           kind="ExternalInput")
    p_d = nc.dram_tensor("p_d", [128, T_E, B], DT16, kind="ExternalInput")
    relgT = nc.dram_tensor("relgT", [128, B], DT16, kind="ExternalInput")
    identd = nc.dram_tensor("identd", [128, 128], DT16, kind="ExternalInput")
    outd = nc.dram_tensor("out", [1, B], F32, kind="ExternalOutput")

    z_d = nc.dram_tensor("z_d", [N_cap, DIM], DT16)
    ard_in = nc.dram_tensor("ard_in", [128, B], DT16)
    ard_out = nc.dram_tensor("ard_out", [128, B], DT16, addr_space="Shared")

    with tile.TileContext(nc) as tc:
        with (
            tc.tile_pool(name="pers", bufs=1) as sm,
            tc.tile_pool(name="ps", bufs=4, space="PSUM") as ps,
            tc.tile_pool(name="dtp", bufs=1, space="PSUM") as dtp,
            tc.tile_pool(name="ph1", bufs=3) as bg,
            tc.tile_pool(name="ph2", bufs=1) as bg2,
        ):
            nc.gpsimd.load_library(library_config.mlp)

            def load(pool, name, dram, shape, dtype=F32):
                t = pool.tile(shape, dtype, tag=name, name=name)
                nc.sync.dma_start(out=t[:], in_=dram[:])
                return t

            gidx_t = load(sm, "gidx", gidx, [128, N_cap // 16], I16)
            ident16 = load(sm, "identd", identd, [128, 128], DT16)
            tcat_t = load(sm, "tcat", tcat, [128, C_n, 3], DT16)
            aux_t = load(sm, "aux", aux, [128, 4 * SLOTS])
            w_sb = load(sm, "w_sb", wloc, [128, SLOTS, DIM], DT16)
            egidx_t = load(sm, "egidx", egidx, [128, T_E * 128 // 16], I16)
            relgT_t = load(sm, "relgT", relgT, [128, B], DT16)
            p_sb = load(sm, "p_sb", p_d, [128, T_E, B], DT16)
            ones = sm.tile([128, 1], DT16)
            nc.vector.memset(ones[:], 1.0)
            xT = sm.tile([128, N_cap], DT16)
            znT = sm.tile([128, N_cap], DT16)
            stats6 = sm.tile([128, SLOTS, 6], F32)
            nc.vector.memset(stats6[:], 0.0)
            if NCOLS < N_cap:
                nc.vector.memset(znT[:, NCOLS:N_cap], 0.0)

            # ---- BN coefficient + pass-2 helpers (batched over slots) ----
            sc = [sm.tile([128, SLOTS], F32, tag=f"sc{i}", name=f"sc{i}")
                  for i in range(6)]
            a_t = sm.tile([128, SLOTS], F32, tag="a_t")
            c_t = sm.tile([128, SLOTS], F32, tag="c_t")
            V = nc.vector

            def coefs_pass2(b0, b1):
                sl = slice(b0, b1)
                ce, me, ve = (stats6[:, sl, k] for k in (0, 1, 2))
                co, mo, vo = (stats6[:, sl, k] for k in (3, 4, 5))
                invcnt = aux_t[:, b0:b1]
                onem = aux_t[:, SLOTS + b0:SLOTS + b1]
                gT_u = aux_t[:, 2 * SLOTS + b0:2 * SLOTS + b1]
                ubb = aux_t[:, 3 * SLOTS + b0:3 * SLOTS + b1]
                te, to_, s1, s2, mean, var = (t[:, sl] for t in sc)
                V.tensor_tensor(out=te, in0=ce, in1=me, op=OP.mult)
                V.tensor_tensor(out=to_, in0=co, in1=mo, op=OP.mult)
                V.tensor_tensor(out=s1, in0=te, in1=to_, op=OP.add)
                V.tensor_tensor(out=s2, in0=ve, in1=vo, op=OP.add)
                V.tensor_tensor(out=te, in0=te, in1=me, op=OP.mult)
                V.tensor_tensor(out=s2, in0=s2, in1=te, op=OP.add)
                V.tensor_tensor(out=to_, in0=to_, in1=mo, op=OP.mult)
                V.tensor_tensor(out=s2, in0=s2, in1=to_, op=OP.add)
                V.tensor_tensor(out=mean, in0=s1, in1=invcnt, op=OP.mult)
                V.tensor_tensor(out=s2, in0=s2, in1=invcnt, op=OP.mult)
                V.tensor_tensor(out=var, in0=mean, in1=mean, op=OP.mult)
                V.tensor_tensor(out=var, in0=s2, in1=var, op=OP.subtract)
                V.tensor_scalar(out=var, in0=var, scalar1=0.0,
                                scalar2=BN_EPS, op0=OP.max, op1=OP.add)
                nc.scalar.activation(out=var, in_=var, func=AF.Sqrt)
                V.reciprocal(out=var, in_=var)  # := 1/sqrt(var+eps)
                V.tensor_tensor(out=te, in0=gT_u, in1=var, op=OP.mult)
                V.tensor_tensor(out=a_t[:, sl], in0=te, in1=onem, op=OP.add)
                V.tensor_tensor(out=to_, in0=mean, in1=te, op=OP.mult)
                V.tensor_tensor(out=c_t[:, sl], in0=ubb, in1=to_,
                                op=OP.subtract)
                for s in range(b0, b1):
                    a, bb = int(col_start[s]), int(col_start[s + 1])
                    if a == bb:
                        continue
                    zp = ps.tile([128, 512], F32, tag="ps", space="PSUM",
                                 name="zp2")
                    nc.tensor.matmul(zp[:, : bb - a], lhsT=w_sb[:, s, :],
                                     rhs=xT[:, a:bb], start=True, stop=True)
                    nc.scalar.activation(out=znT[:, a:bb], in_=zp[:, : bb - a],
                                         func=AF.Relu, scale=a_t[:, s:s + 1],
                                         bias=c_t[:, s:s + 1])

            # ---- phase 1: pipelined groups of GRP chunks:
            #      gather -> time-embedding -> PE transpose -> xT
            #      + GEMM pass 1 / bn_stats for slots as they complete;
            #      coefs + pass 2 fire per batch once stats are in ----
            GRP = 8
            next_s = 0  # next slot awaiting pass-1 stats
            next_p2 = 0  # next slot awaiting coefs/pass-2
            P2B = (SLOTS + 1) // 2

            def pass1_upto(chi):
                nonlocal next_s, next_p2
                while next_s < SLOTS and int(col_start[next_s + 1]) <= chi:
                    s = next_s
                    a, bb = int(col_start[s]), int(col_start[s + 1])
                    next_s += 1
                    if a == bb:
                        continue
                    zp = ps.tile([128, 512], F32, tag="ps", space="PSUM",
                                 name="zp")
                    nc.tensor.matmul(zp[:, : bb - a], lhsT=w_sb[:, s, :],
                                     rhs=xT[:, a:bb], start=True, stop=True)
                    nc.vector.bn_stats(stats6[:, s, :], zp[:, : bb - a])
                while next_s - next_p2 >= P2B:
                    coefs_pass2(next_p2, next_p2 + P2B)
                    next_p2 += P2B

            for c0 in range(0, C_n, GRP):
                c1 = min(c0 + GRP, C_n)
                w = c1 - c0
                gb = bg.tile([128, GRP, 384], DT16, tag="gb", name="gb")
                xs = bg.tile([128, GRP, 96], DT16, tag="xs", name="xs")
                nc.gpsimd.dma_gather(
                    out_ap=gb[:, :w, :], in_ap=tbl[:],
                    idxs_ap=gidx_t[:, c0 * 8:c1 * 8],
                    num_idxs=w * 128, num_idxs_reg=w * 128,
                    elem_size=384, single_packet=True)
                f4 = gb[:, :w, 96:192].rearrange("p c (k e) -> p c k e", k=3)
                x4 = xs[:, :w, :].rearrange("p c (k e) -> p c k e", k=3)
                t4 = tcat_t[:, c0:c1, :].unsqueeze(3).to_broadcast(
                    [128, w, 3, T_DIM])
                nc.vector.tensor_tensor(out=x4, in0=f4, in1=t4, op=OP.mult)
                nc.vector.tensor_tensor(out=xs[:, :w, :], in0=xs[:, :w, :],
                                        in1=gb[:, :w, 192:288], op=OP.add)
                nc.scalar.activation(out=xs[:, :w, :], in_=xs[:, :w, :],
                                     func=AF.Sin)
                nc.vector.tensor_tensor(out=xs[:, :w, :], in0=xs[:, :w, :],
                                        in1=gb[:, :w, 288:384], op=OP.mult)
                nc.vector.tensor_tensor(out=gb[:, :w, 96:128],
                                        in0=xs[:, :w, 0:32],
                                        in1=xs[:, :w, 32:64], op=OP.add)
                nc.vector.tensor_tensor(out=gb[:, :w, 96:128],
                                        in0=gb[:, :w, 96:128],
                                        in1=xs[:, :w, 64:96], op=OP.add)
                for c in range(c0, c1):
                    pt = ps.tile([128, 128], DT16, tag="ps", space="PSUM",
                                 name="pt")
                    nc.tensor.transpose(out=pt[:], in_=gb[:, c - c0, 0:128],
                                        identity=ident16[:])
                    if c % 2 == 0:
                        nc.vector.tensor_copy(
                            out=xT[:, c * 128:(c + 1) * 128], in_=pt[:])
                    else:
                        nc.scalar.copy(out=xT[:, c * 128:(c + 1) * 128],
                                       in_=pt[:])
                pass1_upto(c1 * 128)

            # ---- phase 2: remaining coefs + pass 2 ----
            if next_p2 < SLOTS:
                coefs_pass2(next_p2, SLOTS)

            # ---- phase 3b: znT -> transpose -> z_d rows, with edge-tile
            #      gathers + fused pool/select matmuls fired behind the
            #      z_d write watermark ----
            dt0 = dtp.tile([128, 512], F32, tag="dt0", space="PSUM",
                           name="dt0")
            dt1 = dtp.tile([128, 512], F32, tag="dt1", space="PSUM",
                           name="dt1")
            GP = 4
            state = {"wc": 0, "et": 0}

            def edges_upto(final=False):
                while state["et"] < T_E:
                    t0 = state["et"]
                    t1 = t0
                    while (t1 < T_E and t1 - t0 < GP
                           and tile_ready[t1] <= state["wc"]):
                        t1 += 1
                    if t1 == t0 or (t1 - t0 < GP and not final):
                        break
                    wt = t1 - t0
                    zg = bg2.tile([128, GP, DIM], DT16, tag="zg", name="zg",
                                  bufs=3)
                    nc.gpsimd.dma_gather(
                        out_ap=zg[:, :wt, :],
                        in_ap=z_d[0:state["wc"] * 128, :],
                        idxs_ap=egidx_t[:, t0 * 8:t1 * 8],
                        num_idxs=wt * 128, num_idxs_reg=wt * 128,
                        elem_size=DIM, single_packet=True)
                    for t in range(t0, t1):
                        nc.tensor.matmul(dt0[:], lhsT=zg[:, t - t0, :],
                                         rhs=p_sb[:, t, 0:512],
                                         start=(t == 0), stop=(t == T_E - 1))
                        nc.tensor.matmul(dt1[:], lhsT=zg[:, t - t0, :],
                                         rhs=p_sb[:, t, 512:B],
                                         start=(t == 0), stop=(t == T_E - 1))
                    state["et"] = t1

            for c0 in range(0, C_n, GRP):
                c1 = min(c0 + GRP, C_n)
                zr = bg2.tile([128, GRP, DIM], DT16, tag="zr", name="zr",
                              bufs=3)
                for c in range(c0, c1):
                    pt = ps.tile([128, 128], DT16, tag="ps", space="PSUM",
                                 name="pt2")
                    nc.tensor.transpose(out=pt[:],
                                        in_=znT[:, c * 128:(c + 1) * 128],
                                        identity=ident16[:])
                    if c % 2 == 0:
                        nc.scalar.copy(out=zr[:, c - c0, :], in_=pt[:])
                    else:
                        nc.vector.tensor_copy(out=zr[:, c - c0, :], in_=pt[:])
                nc.sync.dma_start(
                    out=z_d[c0 * 128:c1 * 128, :].rearrange(
                        "(c p) d -> p c d", p=128),
                    in_=zr[:, : c1 - c0, :])
                state["wc"] = c1
                edges_upto()
            edges_upto(final=True)
            assert state["et"] == T_E

            # ---- phase 4: AllReduce of D^T partials (relgT/NC folded) ----
            dts = bg2.tile([128, B], DT16, tag="dts")
            nc.scalar.copy(out=dts[:, 0:512], in_=dt0[:])
            nc.vector.tensor_copy(out=dts[:, 512:B], in_=dt1[:])
            nc.vector.tensor_tensor(out=dts[:], in0=dts[:], in1=relgT_t[:],
                                    op=OP.add)
            nc.sync.dma_start(out=ard_in[:], in_=dts[:])
            nc.gpsimd.collective_compute(
                "AllReduce", OP.add,
                replica_groups=[list(range(NC))],
                ins=[ard_in.ap().opt()], outs=[ard_out.ap().opt()])

            # ---- phase 5: scoring ----
            v_t = bg2.tile([128, B], DT16, tag="v_t")
            nc.sync.dma_start(out=v_t[:], in_=ard_out[:])
            sq = bg2.tile([128, B], DT16, tag="sq")
            nc.vector.tensor_tensor(out=sq[:], in0=v_t[:], in1=v_t[:],
                                    op=OP.mult)
            ss0 = dtp.tile([1, 512], F32, tag="ss0", space="PSUM", name="ss0")
            ss1 = dtp.tile([1, 512], F32, tag="ss1", space="PSUM", name="ss1")
            nc.tensor.matmul(ss0[:], lhsT=ones[:], rhs=sq[:, 0:512],
                             start=True, stop=True)
            nc.tensor.matmul(ss1[:], lhsT=ones[:], rhs=sq[:, 512:B],
                             start=True, stop=True)
            souts = bg2.tile([1, B], F32, tag="souts")
            nc.scalar.activation(out=souts[:, 0:512], in_=ss0[:], func=AF.Sqrt)
            nc.scalar.activation(out=souts[:, 512:B], in_=ss1[:], func=AF.Sqrt)
            nc.vector.tensor_scalar_mul(souts[:], souts[:], -1.0)
            nc.sync.dma_start(out=outd[:], in_=souts[:])
    nc.finalize()
    return nc


_CACHE = {}


def _in_maps(p):
    return [{
        "tbl": p.tbl,
        "gidx": p.gidx[c],
        "tcat": p.tcat[c],
        "wloc": p.wloc[c],
        "aux": p.aux[c],
        "egidx": p.egidx[c],
        "p_d": p.P[c],
        "relgT": p.relgT,
        "identd": p.ident,
    } for c in range(NC)]


def kernel(**inputs) -> np.ndarray:
    p = _make_plan(inputs)
    key = (p.SLOTS, tuple(p.caps.tolist()), p.NCOLS, p.N_cap, p.C_n, p.T_E,
           p.tile_ready)
    if key not in _CACHE:
        _CACHE[key] = _build(p.SLOTS, p.caps, p.col_start, p.NCOLS, p.N_cap,
                             p.C_n, p.T_E, p.tile_ready)
    nc = _CACHE[key]
    res = run_bass_kernel_spmd(nc, _in_maps(p), core_ids=list(range(NC)))
    return np.ascontiguousarray(res.results[0]["out"]).reshape(B).astype(np.float32)


# revision 36
# speedup vs baseline: 1.2355x; 1.2355x over previous
"""Trainium2 Bass kernel for nn_DE_TGraph (diachronic temporal-graph GNN layer).

Strategy (8 NeuronCores, SPMD, relation-sharded):
  - 460 relations partitioned across 8 cores by size-rank snake order
    (rank group g -> slot g; slot capacity = r16(max count in group), so
    pad waste is small and the slot structure is uniform across cores).
    Per-relation BatchNorm is core-local (no collectives for BN stats).
  - Per-neighbor features gathered with ONE dma_gather from a host-packed
    table [ent(96) | yf mf df | yp mp dp | ya ma da] (+ zero row); within
    each slot neighbors are sorted by entity id for HBM locality.
  - Time embedding on DVE/ACT (sin on ScalarE), x tiles transposed on PE.
  - GEMM: two passes over slots (stationary W per slot). Pass 1 feeds
    bn_stats (per-slot mean/var); pass 2 recomputes z and applies the
    per-relation BN affine + ReLU via one ScalarE activation per slot
    (scale/bias are per-partition columns), writing z^T directly to SBUF.
  - Pooling + scoring fused: only edges whose dst is referenced by
    head_pos/tail_pos are kept (~41%).  Host builds per-128-edge-tile
    matrices P[e,b] = (1[dst_e==head_b] - 1[dst_e==tail_b]) / pcnt[dst_e].
    Device: z rows -> DRAM, dma_gather by edge src, then
    D^T[d,b] += zg_t^T @ P_t accumulated in PSUM.  D^T is each core's
    partial of emb[head]-emb[tail] for all B triples.
  - ONE small AllReduce (8 cores) of D^T [128, B] bf16 (256 KB) with a
    Shared-addr-space output (fast HBM-HBM collective path).
  - Scoring: v = D^T + rel_embs[rels]^T (host-gathered, transposed);
    score = -sqrt(sum_d v^2) via square + ones-vector matmul reduction
    over partitions; output written as [1, B].
"""
import numpy as np

import concourse.bacc as bacc
import concourse.mybir as mybir
import concourse.tile as tile
from concourse import library_config
from concourse.bass_utils import run_bass_kernel_spmd

F32 = mybir.dt.float32
BF16 = mybir.dt.bfloat16
USE_BF16 = True
DT16 = BF16 if USE_BF16 else F32
import ml_dtypes
NPDT16 = ml_dtypes.bfloat16 if USE_BF16 else np.float32
I16 = mybir.dt.int16
AF = mybir.ActivationFunctionType
OP = mybir.AluOpType

NUM_ENT = 10000
NUM_REL = 230
R2 = 2 * NUM_REL
S_DIM = 96
T_DIM = 32
DIM = 128
N = 32768
Q = 4096
E = 32768
B = 1024
NC = 8
BN_EPS = 1e-5


def _r16(x):
    return (int(x) + 15) // 16 * 16


def _r128(x):
    return (int(x) + 127) // 128 * 128


def _wrap16(idx, n):
    """[n] int -> [128, n//16] int16 (index i at [i%16, i//16], replicated)."""
    assert n % 16 == 0 and len(idx) == n
    w = np.asarray(idx).reshape(n // 16, 16).T.astype(np.int16)
    return np.ascontiguousarray(np.tile(w, (8, 1)))


def _tile_rows(x, p=128):
    """[n, d] -> [128, n//p, d] tile layout (row c*128+p -> [p, c, :])."""
    n, d = x.shape
    assert n % p == 0
    return np.ascontiguousarray(x.reshape(n // p, p, d).transpose(1, 0, 2))


class _Plan:
    pass


def _make_plan(inp):
    p = _Plan()
    rel = np.asarray(inp["rel_id"]).astype(np.int64).reshape(-1)
    nidx = np.asarray(inp["neighbor_idx"]).astype(np.int64).reshape(-1)
    years = np.asarray(inp["years"], np.float32).reshape(-1)
    months = np.asarray(inp["months"], np.float32).reshape(-1)
    days = np.asarray(inp["days"], np.float32).reshape(-1)
    psrc = np.asarray(inp["pool_src"]).astype(np.int64).reshape(-1)
    pdst = np.asarray(inp["pool_dst"]).astype(np.int64).reshape(-1)
    head = np.asarray(inp["head_pos"]).astype(np.int64).reshape(-1)
    tail = np.asarray(inp["tail_pos"]).astype(np.int64).reshape(-1)
    rels = np.asarray(inp["rels"]).astype(np.int64).reshape(-1)
    W = np.asarray(inp["W"], np.float32)
    b = np.asarray(inp["b"], np.float32)
    gamma = np.asarray(inp["gamma"], np.float32)
    beta = np.asarray(inp["beta"], np.float32)

    # ---- relation partition: size-rank snake across cores ----
    cnts = np.bincount(rel, minlength=R2)
    order = np.argsort(-cnts, kind="stable")
    SLOTS = (R2 + NC - 1) // NC
    core_rels = [[None] * SLOTS for _ in range(NC)]
    caps = np.zeros(SLOTS, np.int64)
    for s in range(SLOTS):
        grp = order[s * NC : (s + 1) * NC]
        caps[s] = cnts[grp[0]]  # exact group max; only the total is padded
        perm = range(NC) if s % 2 == 0 else range(NC - 1, -1, -1)
        for r, c in zip(grp, perm):
            core_rels[c][s] = int(r)
    assert caps.max() <= 512, "relation bucket exceeds 512 (unsupported)"
    col_start = np.concatenate([[0], np.cumsum(caps)])
    NCOLS = int(col_start[-1])
    N_cap = _r128(NCOLS)
    C_n = N_cap // 128

    # positions per relation
    order_by_rel = np.argsort(rel, kind="stable")
    rel_sorted = rel[order_by_rel]
    starts = np.searchsorted(rel_sorted, np.arange(R2))
    ends = np.searchsorted(rel_sorted, np.arange(R2), side="right")

    pos_core = np.empty(N, np.int64)
    pos_col = np.empty(N, np.int64)
    gidx_cores, tcat_cores = [], []
    for c in range(NC):
        gidx = np.full(N_cap, NUM_ENT, np.int64)  # zero row
        tc3 = np.zeros((N_cap, 3), np.float32)
        for s in range(SLOTS):
            r = core_rels[c][s]
            if r is None:
                continue
            pp = order_by_rel[starts[r] : ends[r]]
            pp = pp[np.argsort(nidx[pp], kind="stable")]  # HBM locality
            j0 = int(col_start[s])
            gidx[j0 : j0 + len(pp)] = nidx[pp]
            tc3[j0 : j0 + len(pp), 0] = years[pp]
            tc3[j0 : j0 + len(pp), 1] = months[pp]
            tc3[j0 : j0 + len(pp), 2] = days[pp]
            pos_core[pp] = c
            pos_col[pp] = j0 + np.arange(len(pp))
        gidx_cores.append(_wrap16(gidx, N_cap))
        tcat_cores.append(_tile_rows(tc3).astype(NPDT16))

    # ---- per-core weights + BN aux ----
    wloc_cores, aux_cores = [], []
    for c in range(NC):
        wl = np.zeros((SLOTS, DIM, DIM), np.float32)
        invcnt = np.ones(SLOTS, np.float32)
        onem = np.ones(SLOTS, np.float32)
        gT_u = np.zeros((DIM, SLOTS), np.float32)
        ubb = np.zeros((DIM, SLOTS), np.float32)
        for s in range(SLOTS):
            r = core_rels[c][s]
            if r is None:
                continue
            wl[s] = W[r]
            cnt = cnts[r]
            invcnt[s] = 1.0 / max(cnt, 1)
            u = 1.0 if cnt > 1 else 0.0
            onem[s] = 1.0 - u
            gT_u[:, s] = gamma[r] * u
            ubb[:, s] = beta[r] * u + b[r] * (1.0 - u)
        aux = np.zeros((128, 4 * SLOTS), np.float32)
        aux[:, 0:SLOTS] = invcnt[None, :]
        aux[:, SLOTS : 2 * SLOTS] = onem[None, :]
        aux[:, 2 * SLOTS : 3 * SLOTS] = gT_u
        aux[:, 3 * SLOTS : 4 * SLOTS] = ubb
        wloc_cores.append(np.ascontiguousarray(
            wl.transpose(1, 0, 2).astype(NPDT16)))
        aux_cores.append(np.ascontiguousarray(aux))

    # ---- pooling edges: keep only dsts referenced by head/tail ----
    pcnt = np.bincount(pdst, minlength=Q).astype(np.float32)
    used = np.zeros(Q, bool)
    used[head] = True
    used[tail] = True
    keep = used[pdst]
    e_core = pos_core[psrc]
    ecols, edsts = [], []
    for c in range(NC):
        m = keep & (e_core == c)
        es, ed = psrc[m], pdst[m]
        o = np.argsort(pos_col[es], kind="stable")  # HBM locality
        ecols.append(pos_col[es[o]])
        edsts.append(ed[o])
    T_E = max(1, max((len(x) + 127) // 128 for x in ecols))
    NE = T_E * 128

    egidx_cores, p_cores = [], []
    tile_ready = np.zeros(T_E, np.int64)  # z_d chunks needed per edge tile
    for c in range(NC):
        ec, ed = ecols[c], edsts[c]
        L = len(ec)
        eg = np.zeros(NE, np.int64)
        eg[:L] = ec
        dl = np.full(NE, -1, np.int64)
        dl[:L] = ed
        inv = np.zeros(NE, np.float32)
        inv[:L] = 1.0 / np.maximum(pcnt[ed], 1.0)
        P = ((dl[:, None] == head[None, :]).astype(np.float32)
             - (dl[:, None] == tail[None, :]).astype(np.float32))
        P *= inv[:, None]
        egidx_cores.append(_wrap16(eg, NE))
        p_cores.append(np.ascontiguousarray(
            P.reshape(T_E, 128, B).transpose(1, 0, 2).astype(NPDT16)))
        hi = eg.reshape(T_E, 128).max(axis=1)  # cols sorted -> per-tile max
        tile_ready = np.maximum(tile_ready, hi // 128 + 1)
    p.tile_ready = tuple(int(x) for x in tile_ready)

    # ---- scoring: rel_embs[rels]^T / NC, folded pre-AllReduce ----
    relgT = np.asarray(inp["rel_embs"], np.float32)[rels].T / NC  # [128, B]
    p.relgT = np.ascontiguousarray(relgT.astype(NPDT16))
    p.ident = np.ascontiguousarray(np.eye(128, dtype=NPDT16))

    # ---- packed entity/time table (replicated) ----
    tbl = np.zeros((NUM_ENT + 1, 384), np.float32)
    tbl[:NUM_ENT, 0:96] = np.asarray(inp["ent_embs"], np.float32)
    for k, nm in enumerate(["y_freq", "m_freq", "d_freq",
                            "y_phi", "m_phi", "d_phi",
                            "y_amp", "m_amp", "d_amp"]):
        tbl[:NUM_ENT, 96 + 32 * k : 128 + 32 * k] = np.asarray(inp[nm], np.float32)
    p.tbl = np.ascontiguousarray(tbl.astype(NPDT16))

    p.SLOTS, p.caps, p.col_start = SLOTS, caps, col_start
    p.NCOLS, p.N_cap, p.C_n, p.T_E = NCOLS, N_cap, C_n, T_E
    p.gidx, p.tcat = gidx_cores, tcat_cores
    p.wloc, p.aux = wloc_cores, aux_cores
    p.egidx, p.P = egidx_cores, p_cores
    return p


def _build(SLOTS, caps, col_start, NCOLS, N_cap, C_n, T_E, tile_ready):
    nc = bacc.Bacc(None, target_bir_lowering=False, debug=False, num_devices=NC)
    tbl = nc.dram_tensor("tbl", [NUM_ENT + 1, 384], DT16, kind="ExternalInput")
    gidx = nc.dram_tensor("gidx", [128, N_cap // 16], I16, kind="ExternalInput")
    tcat = nc.dram_tensor("tcat", [128, C_n, 3], DT16, kind="ExternalInput")
    wloc = nc.dram_tensor("wloc", [128, SLOTS, DIM], DT16, kind="ExternalInput")
    aux = nc.dram_tensor("aux", [128, 4 * SLOTS], F32, kind="ExternalInput")
    egidx = nc.dram_tensor("egidx", [128, T_E * 128 // 16], I16,
                           kind="ExternalInput")
    p_d = nc.dram_tensor("p_d", [128, T_E, B], DT16, kind="ExternalInput")
    relgT = nc.dram_tensor("relgT", [128, B], DT16, kind="ExternalInput")
    identd = nc.dram_tensor("identd", [128, 128], DT16, kind="ExternalInput")
    outd = nc.dram_tensor("out", [1, B], F32, kind="ExternalOutput")

    z_d = nc.dram_tensor("z_d", [N_cap, DIM], DT16)
    ard_in = nc.dram_tensor("ard_in", [128, B], DT16)
    ard_out = nc.dram_tensor("ard_out", [128, B], DT16, addr_space="Shared")
    war_in = nc.dram_tensor("war_in", [128, 16], DT16)
    war_out = nc.dram_tensor("war_out", [128, 16], DT16, addr_space="Shared")

    with tile.TileContext(nc) as tc:
        with (
            tc.tile_pool(name="pers", bufs=1) as sm,
            tc.tile_pool(name="ps", bufs=6, space="PSUM") as ps,
            tc.tile_pool(name="dtp", bufs=1, space="PSUM") as dtp,
            tc.tile_pool(name="ph1", bufs=3) as bg,
            tc.tile_pool(name="ph2", bufs=1) as bg2,
        ):
            nc.gpsimd.load_library(library_config.mlp)

            def load(pool, name, dram, shape, dtype=F32):
                t = pool.tile(shape, dtype, tag=name, name=name)
                nc.sync.dma_start(out=t[:], in_=dram[:])
                return t

            gidx_t = load(sm, "gidx", gidx, [128, N_cap // 16], I16)
            ident16 = load(sm, "identd", identd, [128, 128], DT16)
            tcat_t = load(sm, "tcat", tcat, [128, C_n, 3], DT16)
            aux_t = load(sm, "aux", aux, [128, 4 * SLOTS])
            ones = sm.tile([128, 1], DT16)
            nc.vector.memset(ones[:], 1.0)
            # warm-up collective: pays the CC firmware spin-up cost and
            # re-synchronizes the cores early, off the critical path
            wu = sm.tile([128, 16], DT16, tag="wu")
            nc.vector.memset(wu[:], 0.0)
            nc.sync.dma_start(out=war_in[:], in_=wu[:])
            nc.gpsimd.collective_compute(
                "AllReduce", OP.add,
                replica_groups=[list(range(NC))],
                ins=[war_in.ap().opt()], outs=[war_out.ap().opt()])
            xT = sm.tile([128, N_cap], DT16)
            znT = sm.tile([128, N_cap], DT16)
            stats6 = sm.tile([128, SLOTS, 6], F32)
            nc.vector.memset(stats6[:], 0.0)
            if NCOLS < N_cap:
                nc.vector.memset(znT[:, NCOLS:N_cap], 0.0)

            # ---- BN coefficient + pass-2 helpers (batched over slots) ----
            sc = [sm.tile([128, SLOTS], F32, tag=f"sc{i}", name=f"sc{i}")
                  for i in range(6)]
            a_t = sm.tile([128, SLOTS], F32, tag="a_t")
            c_t = sm.tile([128, SLOTS], F32, tag="c_t")
            V = nc.vector

            def coefs_pass2(b0, b1):
                sl = slice(b0, b1)
                ce, me, ve = (stats6[:, sl, k] for k in (0, 1, 2))
                co, mo, vo = (stats6[:, sl, k] for k in (3, 4, 5))
                invcnt = aux_t[:, b0:b1]
                onem = aux_t[:, SLOTS + b0:SLOTS + b1]
                gT_u = aux_t[:, 2 * SLOTS + b0:2 * SLOTS + b1]
                ubb = aux_t[:, 3 * SLOTS + b0:3 * SLOTS + b1]
                te, to_, s1, s2, mean, var = (t[:, sl] for t in sc)
                V.tensor_tensor(out=te, in0=ce, in1=me, op=OP.mult)
                V.tensor_tensor(out=to_, in0=co, in1=mo, op=OP.mult)
                V.tensor_tensor(out=s1, in0=te, in1=to_, op=OP.add)
                V.tensor_tensor(out=s2, in0=ve, in1=vo, op=OP.add)
                V.tensor_tensor(out=te, in0=te, in1=me, op=OP.mult)
                V.tensor_tensor(out=s2, in0=s2, in1=te, op=OP.add)
                V.tensor_tensor(out=to_, in0=to_, in1=mo, op=OP.mult)
                V.tensor_tensor(out=s2, in0=s2, in1=to_, op=OP.add)
                V.tensor_tensor(out=mean, in0=s1, in1=invcnt, op=OP.mult)
                V.tensor_tensor(out=s2, in0=s2, in1=invcnt, op=OP.mult)
                V.tensor_tensor(out=var, in0=mean, in1=mean, op=OP.mult)
                V.tensor_tensor(out=var, in0=s2, in1=var, op=OP.subtract)
                V.tensor_scalar(out=var, in0=var, scalar1=0.0,
                                scalar2=BN_EPS, op0=OP.max, op1=OP.add)
                nc.scalar.activation(out=var, in_=var, func=AF.Sqrt)
                V.reciprocal(out=var, in_=var)  # := 1/sqrt(var+eps)
                V.tensor_tensor(out=te, in0=gT_u, in1=var, op=OP.mult)
                V.tensor_tensor(out=a_t[:, sl], in0=te, in1=onem, op=OP.add)
                V.tensor_tensor(out=to_, in0=mean, in1=te, op=OP.mult)
                V.tensor_tensor(out=c_t[:, sl], in0=ubb, in1=to_,
                                op=OP.subtract)
                for s in range(b0, b1):
                    a, bb = int(col_start[s]), int(col_start[s + 1])
                    if a == bb:
                        continue
                    zp = ps.tile([128, 512], F32, tag="ps", space="PSUM",
                                 name="zp2")
                    nc.tensor.matmul(zp[:, : bb - a], lhsT=w_sb[:, s, :],
                                     rhs=xT[:, a:bb], start=True, stop=True)
                    nc.scalar.activation(out=znT[:, a:bb], in_=zp[:, : bb - a],
                                         func=AF.Relu, scale=a_t[:, s:s + 1],
                                         bias=c_t[:, s:s + 1])

            # ---- phase 1: pipelined groups of GRP chunks:
            #      gather -> time-embedding -> PE transpose -> xT
            #      + GEMM pass 1 / bn_stats for slots as they complete;
            #      coefs + pass 2 fire per batch once stats are in ----
            GRP = 8
            next_s = 0  # next slot awaiting pass-1 stats
            next_p2 = 0  # next slot awaiting coefs/pass-2
            P2B = (SLOTS + 1) // 2
            big_loaded = False

            def pass1_upto(chi):
                nonlocal next_s, next_p2
                while next_s < SLOTS and int(col_start[next_s + 1]) <= chi:
                    s = next_s
                    a, bb = int(col_start[s]), int(col_start[s + 1])
                    next_s += 1
                    if a == bb:
                        continue
                    zp = ps.tile([128, 512], F32, tag="ps", space="PSUM",
                                 name="zp")
                    nc.tensor.matmul(zp[:, : bb - a], lhsT=w_sb[:, s, :],
                                     rhs=xT[:, a:bb], start=True, stop=True)
                    nc.vector.bn_stats(stats6[:, s, :], zp[:, : bb - a])
                while next_s - next_p2 >= P2B:
                    coefs_pass2(next_p2, next_p2 + P2B)
                    next_p2 += P2B

            for c0 in range(0, C_n, GRP):
                c1 = min(c0 + GRP, C_n)
                w = c1 - c0
                gb = bg.tile([128, GRP, 384], DT16, tag="gb", name="gb")
                xs = bg.tile([128, GRP, 96], DT16, tag="xs", name="xs")
                nc.gpsimd.dma_gather(
                    out_ap=gb[:, :w, :], in_ap=tbl[:],
                    idxs_ap=gidx_t[:, c0 * 8:c1 * 8],
                    num_idxs=w * 128, num_idxs_reg=w * 128,
                    elem_size=384, single_packet=True)
                if not big_loaded:
                    # big parameter loads, queued behind the first gather's
                    # index read so they don't delay it
                    big_loaded = True
                    w_sb = load(sm, "w_sb", wloc, [128, SLOTS, DIM], DT16)
                    egidx_t = load(sm, "egidx", egidx,
                                   [128, T_E * 128 // 16], I16)
                    relgT_t = load(sm, "relgT", relgT, [128, B], DT16)
                    p_sb = load(sm, "p_sb", p_d, [128, T_E, B], DT16)
                f4 = gb[:, :w, 96:192].rearrange("p c (k e) -> p c k e", k=3)
                x4 = xs[:, :w, :].rearrange("p c (k e) -> p c k e", k=3)
                t4 = tcat_t[:, c0:c1, :].unsqueeze(3).to_broadcast(
                    [128, w, 3, T_DIM])
                nc.vector.tensor_tensor(out=x4, in0=f4, in1=t4, op=OP.mult)
                nc.vector.tensor_tensor(out=xs[:, :w, :], in0=xs[:, :w, :],
                                        in1=gb[:, :w, 192:288], op=OP.add)
                nc.scalar.activation(out=xs[:, :w, :], in_=xs[:, :w, :],
                                     func=AF.Sin)
                nc.vector.tensor_tensor(out=xs[:, :w, :], in0=xs[:, :w, :],
                                        in1=gb[:, :w, 288:384], op=OP.mult)
                nc.vector.tensor_tensor(out=gb[:, :w, 96:128],
                                        in0=xs[:, :w, 0:32],
                                        in1=xs[:, :w, 32:64], op=OP.add)
                nc.vector.tensor_tensor(out=gb[:, :w, 96:128],
                                        in0=gb[:, :w, 96:128],
                                        in1=xs[:, :w, 64:96], op=OP.add)
                for c in range(c0, c1):
                    pt = ps.tile([128, 128], DT16, tag="ps", space="PSUM",
                                 name="pt")
                    nc.tensor.transpose(out=pt[:], in_=gb[:, c - c0, 0:128],
                                        identity=ident16[:])
                    if c % 2 == 0:
                        nc.vector.tensor_copy(
                            out=xT[:, c * 128:(c + 1) * 128], in_=pt[:])
                    else:
                        nc.scalar.copy(out=xT[:, c * 128:(c + 1) * 128],
                                       in_=pt[:])
                pass1_upto(c1 * 128)

            # ---- phase 2: remaining coefs + pass 2 ----
            if next_p2 < SLOTS:
                coefs_pass2(next_p2, SLOTS)

            # ---- phase 3b: znT -> transpose -> z_d rows, with edge-tile
            #      gathers + fused pool/select matmuls fired behind the
            #      z_d write watermark ----
            dt0 = dtp.tile([128, 512], F32, tag="dt0", space="PSUM",
                           name="dt0")
            dt1 = dtp.tile([128, 512], F32, tag="dt1", space="PSUM",
                           name="dt1")
            GP = 4
            state = {"wc": 0, "et": 0}

            def edges_upto(final=False):
                while state["et"] < T_E:
                    t0 = state["et"]
                    t1 = t0
                    while (t1 < T_E and t1 - t0 < GP
                           and tile_ready[t1] <= state["wc"]):
                        t1 += 1
                    if t1 == t0 or (t1 - t0 < GP and not final):
                        break
                    wt = t1 - t0
                    zg = bg2.tile([128, GP, DIM], DT16, tag="zg", name="zg",
                                  bufs=3)
                    nc.gpsimd.dma_gather(
                        out_ap=zg[:, :wt, :],
                        in_ap=z_d[0:state["wc"] * 128, :],
                        idxs_ap=egidx_t[:, t0 * 8:t1 * 8],
                        num_idxs=wt * 128, num_idxs_reg=wt * 128,
                        elem_size=DIM, single_packet=True)
                    for t in range(t0, t1):
                        nc.tensor.matmul(dt0[:], lhsT=zg[:, t - t0, :],
                                         rhs=p_sb[:, t, 0:512],
                                         start=(t == 0), stop=(t == T_E - 1))
                        nc.tensor.matmul(dt1[:], lhsT=zg[:, t - t0, :],
                                         rhs=p_sb[:, t, 512:B],
                                         start=(t == 0), stop=(t == T_E - 1))
                    state["et"] = t1

            for c0 in range(0, C_n, GRP):
                c1 = min(c0 + GRP, C_n)
                zr = bg2.tile([128, GRP, DIM], DT16, tag="zr", name="zr",
                              bufs=3)
                for c in range(c0, c1):
                    pt = ps.tile([128, 128], DT16, tag="ps", space="PSUM",
                                 name="pt2")
                    nc.tensor.transpose(out=pt[:],
                                        in_=znT[:, c * 128:(c + 1) * 128],
                                        identity=ident16[:])
                    if c % 2 == 0:
                        nc.scalar.copy(out=zr[:, c - c0, :], in_=pt[:])
                    else:
                        nc.vector.tensor_copy(out=zr[:, c - c0, :], in_=pt[:])
                nc.sync.dma_start(
                    out=z_d[c0 * 128:c1 * 128, :].rearrange(
                        "(c p) d -> p c d", p=128),
                    in_=zr[:, : c1 - c0, :])
                state["wc"] = c1
                edges_upto()
            edges_upto(final=True)
            assert state["et"] == T_E

            # ---- phase 4: AllReduce of D^T partials (relgT/NC folded) ----
            dts = bg2.tile([128, B], DT16, tag="dts")
            nc.scalar.copy(out=dts[:, 0:512], in_=dt0[:])
            nc.vector.tensor_copy(out=dts[:, 512:B], in_=dt1[:])
            nc.vector.tensor_tensor(out=dts[:], in0=dts[:], in1=relgT_t[:],
                                    op=OP.add)
            nc.sync.dma_start(out=ard_in[:], in_=dts[:])
            nc.gpsimd.collective_compute(
                "AllReduce", OP.add,
                replica_groups=[list(range(NC))],
                ins=[ard_in.ap().opt()], outs=[ard_out.ap().opt()])

            # ---- phase 5: scoring ----
            v_t = bg2.tile([128, B], DT16, tag="v_t")
            nc.sync.dma_start(out=v_t[:], in_=ard_out[:])
            sq = bg2.tile([128, B], DT16, tag="sq")
            nc.vector.tensor_tensor(out=sq[:], in0=v_t[:], in1=v_t[:],
                                    op=OP.mult)
            ss0 = ps.tile([1, 512], F32, tag="ps", space="PSUM", name="ss0")
            ss1 = ps.tile([1, 512], F32, tag="ps", space="PSUM", name="ss1")
            nc.tensor.matmul(ss0[:], lhsT=ones[:], rhs=sq[:, 0:512],
                             start=True, stop=True)
            nc.tensor.matmul(ss1[:], lhsT=ones[:], rhs=sq[:, 512:B],
                             start=True, stop=True)
            souts = bg2.tile([1, B], F32, tag="souts")
            nc.scalar.activation(out=souts[:, 0:512], in_=ss0[:], func=AF.Sqrt)
            nc.scalar.activation(out=souts[:, 512:B], in_=ss1[:], func=AF.Sqrt)
            nc.vector.tensor_scalar_mul(souts[:], souts[:], -1.0)
            nc.sync.dma_start(out=outd[:], in_=souts[:])
    nc.finalize()
    return nc


_CACHE = {}


def _in_maps(p):
    return [{
        "tbl": p.tbl,
        "gidx": p.gidx[c],
        "tcat": p.tcat[c],
        "wloc": p.wloc[c],
        "aux": p.aux[c],
        "egidx": p.egidx[c],
        "p_d": p.P[c],
        "relgT": p.relgT,
        "identd": p.ident,
    } for c in range(NC)]


def kernel(**inputs) -> np.ndarray:
    p = _make_plan(inputs)
    key = (p.SLOTS, tuple(p.caps.tolist()), p.NCOLS, p.N_cap, p.C_n, p.T_E,
           p.tile_ready)
    if key not in _CACHE:
        _CACHE[key] = _build(p.SLOTS, p.caps, p.col_start, p.NCOLS, p.N_cap,
                             p.C_n, p.T_E, p.tile_ready)
    nc = _CACHE[key]
    res = run_bass_kernel_spmd(nc, _in_maps(p), core_ids=list(range(NC)))
    return np.ascontiguousarray(res.results[0]["out"]).reshape(B).astype(np.float32)
